# revision 37
# baseline (speedup 1.0000x reference)
"""Trainium2 Bass kernel for nn_BatchedFCN (batched ensemble MLP + max).

Reference computation (per network n of 1024, batch B=256):
    h = relu(x @ W1_n^T + b1); h = relu(h @ W2_n^T + b2); h = relu(h @ W3_n^T + b3)
    h = relu(h @ W4_n^T + b4); y_n = h @ W5_n^T + b5          # [B, 1]
    out[b] = max_n y_n[b]                                      # [B]

Sharding: the 1024 networks are split across 8 NeuronCores (128 nets/core).
Each core computes a partial max over its networks; the host folds the 8
partial results.

On-chip dataflow (per core): activations live transposed [features, batch]
so each layer is matmul(out=psum, lhsT=W^T (stationary), rhs=h^T (moving)).
Biases are folded into the weights by augmenting each W^T with one extra
input row (the bias) and one extra output column that propagates a row of
ones through the network (relu(1) == 1), so activations are pure relu and
no per-network bias operand is needed.

Layers 4/5 have narrow outputs (50 / 1), so two networks are packed per
matmul "slot" via PE tile positions: net A at array columns 0.., net B at
column offset 64 (L4), and L5 results for 8 networks land in one PSUM bank
at partitions {0,32,64,96} x free halves. A vector tensor_tensor(max)
accumulates the ensemble max; the final fold happens on the host.

All compute in bf16 (fp32 PSUM accumulation); weights are pre-transposed,
augmented, and cast to bf16 on the host.
"""

import sys

import numpy as np

try:
    import concourse  # noqa: F401
except ImportError:  # fall back to the container's staged repo
    sys.path.insert(0, "/opt/trn_rl_repo")

import ml_dtypes  # noqa: E402

import concourse.mybir as mybir  # noqa: E402
import concourse.tile as tile  # noqa: E402
from concourse import bacc, bass_utils  # noqa: E402

# Problem shapes (hardcoded per contract)
NN = 1024  # total networks
B = 256  # batch
NCORES = 8
NPC = NN // NCORES  # networks per core = 128
PAIRS = NPC // 2  # 64
GROUPS = 4  # weight-DMA groups per core
GNETS = NPC // GROUPS  # 32 nets per group
GPAIRS = GNETS // 2  # 16 pairs per group

KA = 501  # augmented L1 contraction (500 inputs + bias row)
MA = 101  # augmented hidden width (100 + ones column)
M4 = 51  # augmented layer-4 output (50 + ones column)
M4P = 64  # L4 output padded to a 64-wide PE column group (13 zero columns)
CH = (126, 125, 125, 125)  # L1 K-chunk sizes (sum = 501)
OFF = (0, 126, 251, 376)

BF16 = ml_dtypes.bfloat16

_PROGRAM_CACHE = {}


def _build_program():
    """Build the SPMD Bass program (same program for all 8 cores)."""
    nc = bacc.Bacc("TRN2", debug=False, num_devices=NCORES)
    bf = mybir.dt.bfloat16
    f32 = mybir.dt.float32

    xp_d = nc.dram_tensor("xp", [128, 4 * B], bf, kind="ExternalInput").ap()
    w1_d = nc.dram_tensor("w1p", [128, NPC * 4 * MA], bf, kind="ExternalInput").ap()
    w2_d = nc.dram_tensor("w2p", [MA, NPC * MA], bf, kind="ExternalInput").ap()
    w3_d = nc.dram_tensor("w3p", [MA, NPC * MA], bf, kind="ExternalInput").ap()
    w4_d = nc.dram_tensor("w4p", [MA, NPC * M4P], bf, kind="ExternalInput").ap()
    w5_d = nc.dram_tensor("w5p", [128, PAIRS], bf, kind="ExternalInput").ap()
    out_d = nc.dram_tensor("out", [4, 512], f32, kind="ExternalOutput").ap()

    relu = mybir.ActivationFunctionType.Relu

    with tile.TileContext(nc) as tc:
        from contextlib import ExitStack

        with ExitStack() as ctx:
            consts = ctx.enter_context(tc.tile_pool(name="consts", bufs=1))
            wp1 = ctx.enter_context(tc.tile_pool(name="wp1", bufs=2))
            wp2 = ctx.enter_context(tc.tile_pool(name="wp2", bufs=2))
            wp3 = ctx.enter_context(tc.tile_pool(name="wp3", bufs=2))
            wp4 = ctx.enter_context(tc.tile_pool(name="wp4", bufs=2))
            hp = ctx.enter_context(tc.tile_pool(name="hp", bufs=4))
            pp1 = ctx.enter_context(tc.tile_pool(name="pp1", bufs=2, space="PSUM"))
            pp2 = ctx.enter_context(tc.tile_pool(name="pp2", bufs=2, space="PSUM"))
            pp3 = ctx.enter_context(tc.tile_pool(name="pp3", bufs=1, space="PSUM"))
            pp4 = ctx.enter_context(tc.tile_pool(name="pp4", bufs=1, space="PSUM"))
            pp5 = ctx.enter_context(tc.tile_pool(name="pp5", bufs=1, space="PSUM"))

            # xp/w5t go on the ACT HWDGE ring so they don't queue ahead of the
            # first w1 chunk on the SP ring
            xp = consts.tile([128, 4 * B], bf)
            nc.scalar.dma_start(xp, xp_d)
            w5t = consts.tile([128, PAIRS], bf)
            nc.scalar.dma_start(w5t, w5_d)
            acc = consts.tile([128, 512], f32)
            nc.vector.memset(acc, -3.0e38)
            # two L5 accumulation banks: row-group-0 results (A nets) and
            # row-group-64 results (B nets) must land in different PSUM banks
            p5a = pp5.tile([128, 512], f32)
            nc.vector.memset(p5a, 0.0)
            p5b = pp5.tile([128, 512], f32)
            nc.vector.memset(p5b, 0.0)
            # trigger the one-time ACT table load while the first weight DMA
            # is still in flight
            warm = consts.tile([1, 2], f32)
            nc.vector.memset(warm, 0.0)
            nc.scalar.activation(warm[0:1, 1:2], warm[0:1, 0:1], relu)

            # Software pipeline over pairs: stage skews
            #   L1@0  act1@1  L2,relu2@2  L3,act3@3  L4,relu4@4  L5,fold@5
            # Every instruction's producers complete in an earlier step (or
            # earlier in the same step on an already-running engine), so each
            # engine streams its per-step work without round-trip stalls.
            group_tiles = {}
            p1_t, p2_t, p3_t, p4_t = {}, {}, {}, {}
            h1_t, h2_t, h3_t, h4_t = {}, {}, {}, {}

            SKEW_MAX = 5
            DMA_LEAD = GPAIRS // 2  # steps of lead time for group weight DMAs
            for t in range(PAIRS + SKEW_MAX):
                # weight DMAs, prefetched DMA_LEAD steps ahead of first use
                tl = t + DMA_LEAD
                if t == 0 or (tl % GPAIRS == 0 and tl // GPAIRS < GROUPS):
                    g = 0 if t == 0 else tl // GPAIRS
                    w1t = wp1.tile([128, GNETS * 4 * MA], bf, tag="w1")
                    ncol = 4 * MA  # w1 cols per net

                    def w1_chunk(n0, n1, g=g, w1t=w1t):
                        # split the big w1 transfer so the first chunk's
                        # matmuls can start while the rest streams in
                        nc.sync.dma_start(
                            w1t[:, n0 * ncol : n1 * ncol],
                            w1_d[
                                :,
                                (g * GNETS + n0) * ncol : (g * GNETS + n1) * ncol,
                            ],
                        )

                    w1_chunk(0, 2)
                    w1_chunk(2, 4)
                    w2t = wp2.tile([MA, GNETS * MA], bf, tag="w2")
                    nc.sync.dma_start(
                        w2t, w2_d[:, g * GNETS * MA : (g + 1) * GNETS * MA]
                    )
                    w1_chunk(4, 8)
                    w3t = wp3.tile([MA, GNETS * MA], bf, tag="w3")
                    nc.sync.dma_start(
                        w3t, w3_d[:, g * GNETS * MA : (g + 1) * GNETS * MA]
                    )
                    w1_chunk(8, 14)
                    w4t = wp4.tile([MA, GNETS * M4P], bf, tag="w4")
                    nc.sync.dma_start(
                        w4t, w4_d[:, g * GNETS * M4P : (g + 1) * GNETS * M4P]
                    )
                    w1_chunk(14, 23)
                    w1_chunk(23, 32)
                    group_tiles[g] = (w1t, w2t, w3t, w4t)

                def loc(p):
                    # group-local A/B net indices for pair p
                    jj = p % GPAIRS
                    return p // GPAIRS, 2 * jj, 2 * jj + 1

                # ---- PE stage L2 (pair t-2)
                p_ = t - 2
                if 0 <= p_ < PAIRS:
                    g, nA, nB_ = loc(p_)
                    w2t = group_tiles[g][1]
                    h1 = h1_t.pop(p_)
                    p2 = pp2.tile([128, 512], f32, tag="p2")
                    for nl, fo in ((nA, 0), (nB_, B)):
                        nc.tensor.matmul(
                            p2[0:MA, fo : fo + B],
                            lhsT=w2t[:, nl * MA : (nl + 1) * MA],
                            rhs=h1[:, fo : fo + B],
                        )
                    p2_t[p_] = p2

                # ---- PE stage L3 (pair t-3)
                p_ = t - 3
                if 0 <= p_ < PAIRS:
                    g, nA, nB_ = loc(p_)
                    w3t = group_tiles[g][2]
                    h2 = h2_t.pop(p_)
                    p3 = pp3.tile([128, 512], f32, tag="p3")
                    for nl, fo in ((nA, 0), (nB_, B)):
                        nc.tensor.matmul(
                            p3[0:MA, fo : fo + B],
                            lhsT=w3t[:, nl * MA : (nl + 1) * MA],
                            rhs=h2[:, fo : fo + B],
                        )
                    p3_t[p_] = p3

                # ---- PE stage L4 (pair t-4): A at PE cols 0..63, B at 64..127
                p_ = t - 4
                if 0 <= p_ < PAIRS:
                    g, nA, nB_ = loc(p_)
                    w4t = group_tiles[g][3]
                    h3 = h3_t.pop(p_)
                    p4 = pp4.tile([128, 512], f32, tag="p4")
                    nc.tensor.matmul(
                        p4[0:M4P, 0:B],
                        lhsT=w4t[:, nA * M4P : (nA + 1) * M4P],
                        rhs=h3[:, 0:B],
                    )
                    nc.tensor.matmul(
                        p4[64 : 64 + M4P, 0:B],
                        lhsT=w4t[:, nB_ * M4P : (nB_ + 1) * M4P],
                        rhs=h3[:, B : 2 * B],
                    )
                    p4_t[p_] = p4

                # ---- PE stage L5 (pair t-5): pair p -> slot (p%4, (p//4)%2)
                p_ = t - 5
                if 0 <= p_ < PAIRS:
                    h4 = h4_t.pop(p_)
                    pos = p_ % 4
                    fo5 = ((p_ // 4) % 2) * B
                    nc.tensor.matmul(
                        p5a[32 * pos : 32 * pos + 1, fo5 : fo5 + B],
                        lhsT=w5t[0:M4, p_ : p_ + 1],
                        rhs=h4[0:M4, :],
                        tile_position=(0, 32 * pos),
                    )
                    nc.tensor.matmul(
                        p5b[32 * pos : 32 * pos + 1, fo5 : fo5 + B],
                        lhsT=w5t[64 : 64 + M4, p_ : p_ + 1],
                        rhs=h4[64 : 64 + M4, :],
                        tile_position=(64, 32 * pos),
                    )
                    if p_ % 4 == 3:
                        # fold this half-bank (4 pairs x 2 row groups) into the
                        # running max
                        hs = slice(fo5, fo5 + B)
                        nc.vector.tensor_max(acc[:, hs], acc[:, hs], p5a[:, hs])
                        nc.vector.tensor_max(acc[:, hs], acc[:, hs], p5b[:, hs])
                        if p_ == PAIRS - 5:
                            # acc[:, 0:256] is final after the last half-0
                            # fold; ship it while the last pairs finish
                            acc4 = acc.rearrange("(a b) f -> a b f", b=32)[:, 0, :]
                            nc.sync.dma_start(out_d[:, 0:B], acc4[:, 0:B])

                # ---- PE stage L1 (pair t): 4 K-chunks accumulate per net
                p_ = t
                if 0 <= p_ < PAIRS:
                    g, nA, nB_ = loc(p_)
                    w1t = group_tiles[g][0]
                    p1 = pp1.tile([128, 512], f32, tag="p1")
                    for nl, fo in ((nA, 0), (nB_, B)):
                        for k in range(4):
                            nc.tensor.matmul(
                                p1[0:MA, fo : fo + B],
                                lhsT=w1t[
                                    0 : CH[k],
                                    (nl * 4 + k) * MA : (nl * 4 + k + 1) * MA,
                                ],
                                rhs=xp[0 : CH[k], k * B : (k + 1) * B],
                                start=(k == 0),
                                stop=(k == 3),
                            )
                    p1_t[p_] = p1

                # ---- ACT stage act1 (pair t-1)
                p_ = t - 1
                if 0 <= p_ < PAIRS:
                    p1 = p1_t.pop(p_)
                    h1 = hp.tile([MA, 512], bf, tag="h1")
                    nc.scalar.activation(h1, p1[0:MA, :], relu)
                    h1_t[p_] = h1

                # ---- ACT stage act3 (pair t-3, same step as L3)
                p_ = t - 3
                if 0 <= p_ < PAIRS:
                    p3 = p3_t.pop(p_)
                    h3 = hp.tile([MA, 512], bf, tag="h3")
                    nc.scalar.activation(h3, p3[0:MA, :], relu)
                    h3_t[p_] = h3

                # ---- DVE stage relu2 (pair t-2, same step as L2)
                p_ = t - 2
                if 0 <= p_ < PAIRS:
                    p2 = p2_t.pop(p_)
                    h2 = hp.tile([MA, 512], bf, tag="h2")
                    nc.vector.tensor_scalar_max(h2, p2[0:MA, :], 0.0)
                    h2_t[p_] = h2

                # ---- DVE stage relu4 (pair t-4, same step as L4)
                p_ = t - 4
                if 0 <= p_ < PAIRS:
                    p4 = p4_t.pop(p_)
                    h4 = hp.tile([128, B], bf, tag="h4")
                    nc.vector.tensor_scalar_max(h4, p4[:, 0:B], 0.0)
                    h4_t[p_] = h4

            acc4 = acc.rearrange("(a b) f -> a b f", b=32)[:, 0, :]
            nc.sync.dma_start(out_d[:, B : 2 * B], acc4[:, B : 2 * B])

    nc.compile()
    return nc


def _get_program():
    if "nc" not in _PROGRAM_CACHE:
        _PROGRAM_CACHE["nc"] = _build_program()
    return _PROGRAM_CACHE["nc"]


def _pack_inputs(inputs):
    """Host-side: transpose, augment with bias/ones, chunk, cast bf16, shard."""
    x = np.asarray(inputs["x"], np.float32)
    w = {i: np.asarray(inputs[f"w{i}"], np.float32) for i in (1, 2, 3, 4, 5)}
    b = {i: np.asarray(inputs[f"b{i}"], np.float32) for i in (1, 2, 3, 4, 5)}

    # xT' = [x^T ; ones] -> chunked into [128, 4*B]
    xT = np.concatenate([x.T, np.ones((1, B), np.float32)], axis=0)  # [501, 256]
    xp = np.zeros((128, 4 * B), np.float32)
    for k in range(4):
        xp[0 : CH[k], k * B : (k + 1) * B] = xT[OFF[k] : OFF[k] + CH[k], :]
    xp = xp.astype(BF16)

    # W1': [N, 501, 101]; chunk rows -> [128p, N, 4, 101]
    W1 = np.zeros((NN, KA, MA), np.float32)
    W1[:, :500, :100] = w[1].transpose(0, 2, 1)
    W1[:, 500, :100] = b[1]
    W1[:, 500, 100] = 1.0
    w1p = np.zeros((128, NN, 4, MA), np.float32)
    for k in range(4):
        w1p[0 : CH[k], :, k, :] = W1[:, OFF[k] : OFF[k] + CH[k], :].transpose(1, 0, 2)
    w1p = w1p.astype(BF16)

    def aug_mid(wi, bi, mout):
        # -> [101(part=i), N, mout+1]; ones-propagation col + bias row folded in
        A = np.zeros((NN, MA, mout + 1), np.float32)
        A[:, :100, :mout] = wi.transpose(0, 2, 1)
        A[:, 100, :mout] = bi
        A[:, 100, mout] = 1.0
        return A.transpose(1, 0, 2).astype(BF16)

    w2p = aug_mid(w[2], b[2], 100)  # [101, N, 101]
    w3p = aug_mid(w[3], b[3], 100)
    # L4 padded to 64 output columns so the packed matmuls initialize the full
    # [0:128] PSUM partition range that the relu reads
    w4p = np.zeros((NN, MA, M4P), np.float32)
    w4p[:, :100, :50] = w[4].transpose(0, 2, 1)
    w4p[:, 100, :50] = b[4]
    w4p[:, 100, 50] = 1.0
    w4p = w4p.transpose(1, 0, 2).astype(BF16)  # [101, N, 64]

    # W5': per-core [128, 64]; pair column j holds net 2j rows 0..50, net 2j+1 rows 64..114
    w5a = np.zeros((NN, M4), np.float32)
    w5a[:, :50] = w[5][:, 0, :]
    w5a[:, 50] = b[5][:, 0]

    in_maps = []
    for c in range(NCORES):
        sl = slice(c * NPC, (c + 1) * NPC)
        loc5 = w5a[sl]
        w5p = np.zeros((128, PAIRS), np.float32)
        w5p[0:M4, :] = loc5[0::2].T
        w5p[64 : 64 + M4, :] = loc5[1::2].T
        in_maps.append(
            {
                "xp": xp,
                "w1p": np.ascontiguousarray(w1p[:, sl].reshape(128, NPC * 4 * MA)),
                "w2p": np.ascontiguousarray(w2p[:, sl].reshape(MA, NPC * MA)),
                "w3p": np.ascontiguousarray(w3p[:, sl].reshape(MA, NPC * MA)),
                "w4p": np.ascontiguousarray(w4p[:, sl].reshape(MA, NPC * M4P)),
                "w5p": w5p.astype(BF16),
            }
        )
    return in_maps


def _fold_outputs(results):
    r = np.stack([np.asarray(res["out"], np.float32) for res in results])  # [8,4,512]
    r = r.reshape(NCORES, 4, 2, B)
    return np.ascontiguousarray(r.max(axis=(0, 1, 2)).astype(np.float32))


def run(inputs, **run_kwargs):
    """Pack, execute on 8 cores, fold. Returns (output[B], BassKernelResults)."""
    nc = _get_program()
    in_maps = _pack_inputs(inputs)
    res = bass_utils.run_bass_kernel_spmd(
        nc, in_maps, core_ids=list(range(NCORES)), **run_kwargs
    )
    return _fold_outputs(res.results), res


def kernel(**inputs):
    out, _ = run(inputs)
    return out
